# revision 17
# baseline (speedup 1.0000x reference)
"""Trainium2 Bass kernel for ContextualInvertedResidual.

Pure data parallel over batch: 32 samples -> 8 cores x 4 samples.

bf16 PE path (fp32 PSUM). The per-sample stages are software-pipelined with
a 1-sample skew and FUSED at (g, b) granularity so the PE instruction
stream alternates one expand matmul (sample s) with the nine depthwise
diagonal matmuls (sample s-1): the PE never idles waiting for ACT to drain
an expand PSUM bank.

  expand 1x1 (PE, K=64)   -> BN1+ReLU on ACT -> min6 (+context accum) on DVE
  depthwise 3x3: 9 PSUM-accumulating diagonal matmuls per 8-row group on a
                 zero-padded 58-wide spatial layout (taps = free-dim offsets)
  context bias:  tiny matmuls ctx @ w_ctx^T + t2 -> per-partition bias of BN2
  project 1x1:   3 K-accumulating matmuls -> ACT(+t3) -> DVE residual add
"""

import os
import sys

import numpy as np
import ml_dtypes

for _p in ("/opt/trn_rl_repo",):
    if os.path.isdir(_p) and _p not in sys.path:
        sys.path.insert(0, _p)

import concourse.bacc as bacc
import concourse.tile as tile
from concourse import mybir

N_CORES = 8
NS = 4  # samples per core
CIN = 64
CEXP = 384
NB = 3  # channel blocks of 128
H = W = 56
P = H * W  # 3136
PW = 58  # padded row width
PADN = PW * PW + 2  # 3366: +1 lead, +1 tail so all 9 shifted views stay in range
RG = 8  # rows per group
NG = 7  # groups
CH = RG * W  # 448 unpadded chunk
GP = RG * PW  # 464 padded group width
EPS = 1e-5

F32 = mybir.dt.float32
BF16 = mybir.dt.bfloat16
NPBF16 = ml_dtypes.bfloat16

_CACHE = {}

CONFIG = dict(pe=2, pd=3, pp=2, xb=2, hb2=3, outb=2, tmpb=2)


def _build_nc():
    key = tuple(sorted(CONFIG.items()))
    if key in _CACHE:
        return _CACHE[key]

    nc = bacc.Bacc(
        "TRN2", target_bir_lowering=False, debug=False, num_devices=N_CORES
    )

    x_d = nc.dram_tensor("x", [NS, CIN, P], BF16, kind="ExternalInput")
    wexp_d = nc.dram_tensor("wexp", [CIN, CEXP], BF16, kind="ExternalInput")
    dwdiag_d = nc.dram_tensor("dwdiag", [128, NB * 9 * 128], BF16, kind="ExternalInput")
    wctx_d = nc.dram_tensor("wctx", [128, NB * CEXP], BF16, kind="ExternalInput")
    wproj_d = nc.dram_tensor("wproj", [128, NB * CIN], BF16, kind="ExternalInput")
    t1_d = nc.dram_tensor("t1c", [128, NB], F32, kind="ExternalInput")
    t2_d = nc.dram_tensor("t2c", [128, NB], F32, kind="ExternalInput")
    t3_d = nc.dram_tensor("t3c", [CIN, 1], F32, kind="ExternalInput")
    out_d = nc.dram_tensor("out", [NS, CIN, P], BF16, kind="ExternalOutput")

    taps = [(dy, dx) for dy in (-1, 0, 1) for dx in (-1, 0, 1)]

    with tile.TileContext(nc) as tc:
        from contextlib import ExitStack

        with ExitStack() as ctx:
            const = ctx.enter_context(tc.tile_pool(name="const", bufs=1))
            hp = ctx.enter_context(tc.tile_pool(name="hp", bufs=1))
            xp = ctx.enter_context(tc.tile_pool(name="xp", bufs=CONFIG["xb"]))
            hb2p = ctx.enter_context(tc.tile_pool(name="hb2p", bufs=CONFIG["hb2"]))
            outp = ctx.enter_context(tc.tile_pool(name="outp", bufs=CONFIG["outb"]))
            ctxpp = ctx.enter_context(tc.tile_pool(name="ctxpp", bufs=6))
            ctxsp = ctx.enter_context(tc.tile_pool(name="ctxsp", bufs=6))
            b2p = ctx.enter_context(tc.tile_pool(name="b2p", bufs=2))
            tmpp = ctx.enter_context(tc.tile_pool(name="tmpp", bufs=CONFIG["tmpb"]))
            ps_e = ctx.enter_context(tc.tile_pool(name="ps_e", bufs=CONFIG["pe"], space="PSUM"))
            ps_d = ctx.enter_context(tc.tile_pool(name="ps_d", bufs=CONFIG["pd"], space="PSUM"))
            ps_c = ctx.enter_context(tc.tile_pool(name="ps_c", bufs=1, space="PSUM"))
            ps_p = ctx.enter_context(tc.tile_pool(name="ps_p", bufs=CONFIG["pp"], space="PSUM"))

            # ---- constants (wexp/t1 now; the rest deferred past x(0) DMA) ----
            wexp_s = const.tile([128, CEXP], BF16, tag="wexp")
            dwdiag_s = const.tile([128, NB * 9 * 128], BF16, tag="dwdiag")
            wctx_s = const.tile([128, NB * CEXP], BF16, tag="wctx")
            wproj_s = const.tile([128, NB * CIN], BF16, tag="wproj")
            t1_s = const.tile([128, NB], F32, tag="t1")
            t2_s = const.tile([128, NB], F32, tag="t2")
            t3_s = const.tile([CIN, 1], F32, tag="t3")
            nc.sync.dma_start(wexp_s[0:CIN, :], wexp_d[:])
            nc.sync.dma_start(wexp_s[CIN : 2 * CIN, :], wexp_d[:])
            nc.sync.dma_start(t1_s[:], t1_d[:])

            # ---- persistent padded h tiles (borders stay zero forever) ----
            NH = 6
            h_tiles = [
                hp.tile([128, PADN], BF16, tag=f"h{i}", name=f"h{i}")
                for i in range(NH)
            ]
            for t in h_tiles:
                nc.gpsimd.memset(t[:].bitcast(mybir.dt.uint16), 0)

            relu = mybir.ActivationFunctionType.Relu
            ident = mybir.ActivationFunctionType.Identity
            addop = mybir.AluOpType.add
            minop = mybir.AluOpType.min

            state = {}

            def front_pre(s):
                x_t = xp.tile([128, P], BF16, tag="x")
                nc.sync.dma_start(x_t[0:CIN, :], x_d[s])
                nc.sync.dma_start(x_t[CIN : 2 * CIN, :], x_d[s])
                if s == 0:
                    # big weights only needed by back(0), one iteration later
                    nc.sync.dma_start(dwdiag_s[:], dwdiag_d[:])
                    nc.sync.dma_start(wctx_s[:], wctx_d[:])
                    nc.sync.dma_start(wproj_s[:], wproj_d[:])
                    nc.sync.dma_start(t2_s[:], t2_d[:])
                    nc.sync.dma_start(t3_s[:], t3_d[:])
                hts = [h_tiles[(s * NB + b) % NH] for b in range(NB)]
                ctxps = [ctxpp.tile([128, NG], F32, tag="ctxp", name=f"ctxp{b}") for b in range(NB)]
                state[s] = dict(x=x_t, hts=hts, ctxps=ctxps)

            def emit_expand_pair(s, b, g):
                """two K=64 expand matmuls run concurrently on PE row-halves
                (tile_position row tiling): chunk g on rows 0-63, chunk g+1
                on rows 64-127."""
                st = state[s]
                gs = [g] if g + 1 >= NG else [g, g + 1]
                pes = []
                for j, gg in enumerate(gs):
                    pe = ps_e.tile([128, CH], F32, tag="pe", name=f"pe{j}")
                    nc.tensor.matmul(
                        pe[:],
                        wexp_s[j * CIN : (j + 1) * CIN, b * 128 : (b + 1) * 128],
                        st["x"][j * CIN : (j + 1) * CIN, gg * CH : (gg + 1) * CH],
                        start=True,
                        stop=True,
                        tile_position=(j * CIN, 0),
                    )
                    pes.append(pe)
                for gg, pe in zip(gs, pes):
                    dst = (
                        st["hts"][b][:, 60 + GP * gg : 60 + GP * gg + GP]
                        .rearrange("p (r w) -> p r w", w=PW)[:, :, 0:W]
                    )
                    src = pe[:].rearrange("p (r w) -> p r w", w=W)
                    nc.scalar.activation(
                        dst, src, relu, bias=t1_s[:, b : b + 1], scale=1.0
                    )
                    # min(.,6) + context partial sum in one DVE pass
                    nc.vector.tensor_scalar(
                        dst, dst, 6.0, None, minop, addop,
                        accum_out=st["ctxps"][b][:, gg : gg + 1],
                    )

            def front_post(s):
                st = state[s]
                ctxs = []
                with nc.allow_low_precision(reason="bf16 ctx vector is plenty"):
                    for b in range(NB):
                        cs = ctxsp.tile([128, 1], BF16, tag="ctxs")
                        nc.vector.tensor_reduce(
                            cs[:], st["ctxps"][b][:], axis=mybir.AxisListType.X,
                            op=addop,
                        )
                        ctxs.append(cs)
                st["ctxs"] = ctxs

            def back_pre(s):
                st = state[s]
                st["b2"] = b2p.tile([128, NB], F32, tag="b2", name="b2")
                st["hb2s"] = [hb2p.tile([128, P], BF16, tag="hb2", name=f"hb2_{b}") for b in range(NB)]
                out_t = outp.tile([CIN, P], BF16, tag="out")
                st["out"] = out_t

            def emit_ctx(s):
                """context-bias matmuls + BN2 bias vector (bias add on DVE so
                the scalar FIFO never waits on instructions behind it)."""
                st = state[s]
                pc = ps_c.tile([128, NB], F32, tag="pc")
                for chn in range(NB):
                    for kb in range(NB):
                        nc.tensor.matmul(
                            pc[:, chn : chn + 1],
                            wctx_s[:, kb * CEXP + chn * 128 : kb * CEXP + (chn + 1) * 128],
                            st["ctxs"][kb][:],
                            start=(kb == 0),
                            stop=(kb == NB - 1),
                        )
                for chn in range(NB):
                    nc.vector.tensor_scalar_add(
                        st["b2"][:, chn : chn + 1],
                        pc[:, chn : chn + 1],
                        t2_s[:, chn : chn + 1],
                    )

            def emit_dw(s, b, g, defer_act=False):
                st = state[s]
                pd = ps_d.tile([128, GP], F32, tag="pd")
                base = 59 + GP * g
                for t, (dy, dx) in enumerate(taps):
                    off = dy * PW + dx
                    nc.tensor.matmul(
                        pd[:],
                        dwdiag_s[:, (b * 9 + t) * 128 : (b * 9 + t + 1) * 128],
                        st["hts"][b][:, base + off : base + off + GP],
                        start=(t == 0),
                        stop=(t == 8),
                    )

                def bn2():
                    pd_v = pd[:].rearrange("p (r w) -> p r w", w=PW)[:, :, 1 : W + 1]
                    dst = st["hb2s"][b][:, g * CH : (g + 1) * CH].rearrange(
                        "p (r w) -> p r w", w=W
                    )
                    nc.scalar.activation(
                        dst, pd_v, relu, bias=st["b2"][:, b : b + 1], scale=1.0
                    )
                    nc.vector.tensor_scalar_min(dst, dst, 6.0)

                if defer_act:
                    return bn2
                bn2()

            def emit_proj_pair(s, g):
                """project groups g (PE col-half A -> PSUM rows 0-63) and g+1
                (col-half B -> rows 64-127) concurrently via col tiling."""
                st = state[s]
                gs = [g] if g + 1 >= NG else [g, g + 1]
                pp = ps_p.tile([128, CH], F32, tag="pp")
                for kb in range(NB):
                    for j, gg in enumerate(gs):
                        # start=True clears has_written for the WHOLE bank, so
                        # only the first matmul into the bank may carry it
                        nc.tensor.matmul(
                            pp[j * CIN : (j + 1) * CIN, :],
                            wproj_s[:, kb * CIN : (kb + 1) * CIN],
                            st["hb2s"][kb][:, gg * CH : (gg + 1) * CH],
                            start=(kb == 0),
                            stop=(kb == NB - 1),
                            tile_position=(0, j * CIN),
                            skip_group_check=True,
                        )
                for j, gg in enumerate(gs):
                    tmp = tmpp.tile([CIN, CH], BF16, tag="tmp", name=f"tmp{j}")
                    nc.scalar.activation(
                        tmp[:], pp[j * CIN : (j + 1) * CIN, :], ident,
                        bias=t3_s[:], scale=1.0,
                    )
                    nc.vector.tensor_add(
                        st["out"][:, gg * CH : (gg + 1) * CH],
                        tmp[:],
                        st["x"][0:CIN, gg * CH : (gg + 1) * CH],
                    )
                    nc.sync.dma_start(
                        out_d[s][:, gg * CH : (gg + 1) * CH],
                        st["out"][:, gg * CH : (gg + 1) * CH],
                    )

            def back_post(s):
                state.pop(s)

            # fused, skewed pipeline: iteration s interleaves expand(s) with
            # depthwise+project(s-1) at (g, b) granularity
            for s in range(NS + 1):
                if s < NS:
                    front_pre(s)
                if s >= 1:
                    back_pre(s - 1)
                for gp in range(0, NG + 1, 2):
                    for b in range(NB):
                        if s < NS and gp < NG:
                            emit_expand_pair(s, b, gp)
                        first = s >= 1 and gp == 0 and b == 0
                        deferred = []
                        for gg in (gp, gp + 1):
                            if s >= 1 and gg < NG:
                                r = emit_dw(s - 1, b, gg, defer_act=first)
                                if first:
                                    deferred.append(r)
                        if first:
                            # ctx matmuls come after the first dw chains so the
                            # PE is not blocked on sample s-1's full bn1 drain
                            emit_ctx(s - 1)
                            for fn in deferred:
                                fn()
                        if s >= 1 and b == 1 and gp >= 2:
                            emit_proj_pair(s - 1, gp - 2)
                if s >= 1:
                    emit_proj_pair(s - 1, NG - 1)
                if s < NS:
                    front_post(s)
                if s >= 1:
                    back_post(s - 1)

    nc.compile()
    _CACHE[key] = nc
    return nc


def _prep_weights(w_expand, g1, b1, m1, v1, w_dw, w_ctx, g2, b2, m2, v2,
                  w_proj, g3, b3, m3, v3):
    f = np.float32
    s1 = (g1 / np.sqrt(v1 + EPS)).astype(f)
    t1 = (b1 - m1 * s1).astype(f)
    s2 = (g2 / np.sqrt(v2 + EPS)).astype(f)
    t2 = (b2 - m2 * s2).astype(f)
    s3 = (g3 / np.sqrt(v3 + EPS)).astype(f)
    t3 = (b3 - m3 * s3).astype(f)

    wexp = np.ascontiguousarray((w_expand * s1[:, None]).T.astype(f))  # [64, 384]

    wdw = (w_dw[:, 0] * s2[:, None, None]).reshape(CEXP, 9).astype(f)  # [c, t]
    dwdiag = np.zeros((128, NB * 9, 128), f)
    idx = np.arange(128)
    for b in range(NB):
        for t in range(9):
            dwdiag[idx, b * 9 + t, idx] = wdw[b * 128 : (b + 1) * 128, t]
    dwdiag = np.ascontiguousarray(dwdiag.reshape(128, NB * 9 * 128))

    wctx_f = (w_ctx * s2[:, None] / float(P)).astype(f)  # [o, c]
    wctx = np.ascontiguousarray(
        wctx_f.reshape(CEXP, NB, 128).transpose(2, 1, 0).reshape(128, NB * CEXP)
    )

    wproj_f = (w_proj * s3[:, None]).astype(f)  # [64, 384]
    wproj = np.ascontiguousarray(
        wproj_f.reshape(CIN, NB, 128).transpose(2, 1, 0).reshape(128, NB * CIN)
    )

    t1c = np.ascontiguousarray(t1.reshape(NB, 128).T)
    t2c = np.ascontiguousarray(t2.reshape(NB, 128).T)
    t3c = np.ascontiguousarray(t3.reshape(CIN, 1))
    return dict(
        wexp=wexp.astype(NPBF16), dwdiag=dwdiag.astype(NPBF16),
        wctx=wctx.astype(NPBF16), wproj=wproj.astype(NPBF16),
        t1c=t1c, t2c=t2c, t3c=t3c,
    )


def make_in_maps(inputs):
    x = np.asarray(inputs["x"], dtype=np.float32)
    w = _prep_weights(
        np.asarray(inputs["w_expand"], np.float32),
        np.asarray(inputs["g1"], np.float32), np.asarray(inputs["b1"], np.float32),
        np.asarray(inputs["m1"], np.float32), np.asarray(inputs["v1"], np.float32),
        np.asarray(inputs["w_dw"], np.float32),
        np.asarray(inputs["w_ctx"], np.float32),
        np.asarray(inputs["g2"], np.float32), np.asarray(inputs["b2"], np.float32),
        np.asarray(inputs["m2"], np.float32), np.asarray(inputs["v2"], np.float32),
        np.asarray(inputs["w_proj"], np.float32),
        np.asarray(inputs["g3"], np.float32), np.asarray(inputs["b3"], np.float32),
        np.asarray(inputs["m3"], np.float32), np.asarray(inputs["v3"], np.float32),
    )
    in_maps = []
    for c in range(N_CORES):
        shard = np.ascontiguousarray(
            x[c * NS : (c + 1) * NS].reshape(NS, CIN, P).astype(NPBF16)
        )
        in_maps.append({"x": shard, **w})
    return in_maps


def kernel(**inputs):
    from concourse.bass_utils import run_bass_kernel_spmd

    nc = _build_nc()
    in_maps = make_in_maps(inputs)
    res = run_bass_kernel_spmd(nc, in_maps, list(range(N_CORES))).results
    out = np.concatenate(
        [np.asarray(res[c]["out"]).astype(np.float32) for c in range(N_CORES)],
        axis=0,
    )
    return np.ascontiguousarray(out.reshape(32, CIN, H, W))


# revision 18
# speedup vs baseline: 1.0331x; 1.0331x over previous
"""Trainium2 Bass kernel for ContextualInvertedResidual.

Pure data parallel over batch: 32 samples -> 8 cores x 4 samples.

bf16 PE path (fp32 PSUM). The per-sample stages are software-pipelined with
a 1-sample skew and FUSED at (g, b) granularity so the PE instruction
stream alternates one expand matmul (sample s) with the nine depthwise
diagonal matmuls (sample s-1): the PE never idles waiting for ACT to drain
an expand PSUM bank.

  expand 1x1 (PE, K=64)   -> BN1+ReLU on ACT -> min6 (+context accum) on DVE
  depthwise 3x3: 9 PSUM-accumulating diagonal matmuls per 8-row group on a
                 zero-padded 58-wide spatial layout (taps = free-dim offsets)
  context bias:  tiny matmuls ctx @ w_ctx^T + t2 -> per-partition bias of BN2
  project 1x1:   3 K-accumulating matmuls -> ACT(+t3) -> DVE residual add
"""

import os
import sys

import numpy as np
import ml_dtypes

for _p in ("/opt/trn_rl_repo",):
    if os.path.isdir(_p) and _p not in sys.path:
        sys.path.insert(0, _p)

import concourse.bacc as bacc
import concourse.tile as tile
from concourse import mybir

N_CORES = 8
NS = 4  # samples per core
CIN = 64
CEXP = 384
NB = 3  # channel blocks of 128
H = W = 56
P = H * W  # 3136
PW = 58  # padded row width
PADN = PW * PW + 2  # 3366: +1 lead, +1 tail so all 9 shifted views stay in range
RG = 8  # rows per group
NG = 7  # groups
CH = RG * W  # 448 unpadded chunk
GP = RG * PW  # 464 padded group width
EPS = 1e-5

F32 = mybir.dt.float32
BF16 = mybir.dt.bfloat16
NPBF16 = ml_dtypes.bfloat16

_CACHE = {}

CONFIG = dict(pe=3, pd=2, pp=2, xb=2, hb2=3, outb=2, tmpb=2)


def _build_nc():
    key = tuple(sorted(CONFIG.items()))
    if key in _CACHE:
        return _CACHE[key]

    nc = bacc.Bacc(
        "TRN2", target_bir_lowering=False, debug=False, num_devices=N_CORES
    )

    x_d = nc.dram_tensor("x", [NS, CIN, P], BF16, kind="ExternalInput")
    wexp_d = nc.dram_tensor("wexp", [CIN, CEXP], BF16, kind="ExternalInput")
    dwdiag_d = nc.dram_tensor("dwdiag", [128, NB * 9 * 128], BF16, kind="ExternalInput")
    wctx_d = nc.dram_tensor("wctx", [128, NB * CEXP], BF16, kind="ExternalInput")
    wproj_d = nc.dram_tensor("wproj", [128, NB * CIN], BF16, kind="ExternalInput")
    t1_d = nc.dram_tensor("t1c", [128, NB], F32, kind="ExternalInput")
    t2_d = nc.dram_tensor("t2c", [128, NB], F32, kind="ExternalInput")
    t3_d = nc.dram_tensor("t3c", [CIN, 1], F32, kind="ExternalInput")
    out_d = nc.dram_tensor("out", [NS, CIN, P], BF16, kind="ExternalOutput")

    taps = [(dy, dx) for dy in (-1, 0, 1) for dx in (-1, 0, 1)]

    with tile.TileContext(nc) as tc:
        from contextlib import ExitStack

        with ExitStack() as ctx:
            const = ctx.enter_context(tc.tile_pool(name="const", bufs=1))
            hp = ctx.enter_context(tc.tile_pool(name="hp", bufs=1))
            xp = ctx.enter_context(tc.tile_pool(name="xp", bufs=CONFIG["xb"]))
            hb2p = ctx.enter_context(tc.tile_pool(name="hb2p", bufs=CONFIG["hb2"]))
            outp = ctx.enter_context(tc.tile_pool(name="outp", bufs=CONFIG["outb"]))
            ctxpp = ctx.enter_context(tc.tile_pool(name="ctxpp", bufs=6))
            ctxsp = ctx.enter_context(tc.tile_pool(name="ctxsp", bufs=6))
            b2p = ctx.enter_context(tc.tile_pool(name="b2p", bufs=2))
            tmpp = ctx.enter_context(tc.tile_pool(name="tmpp", bufs=CONFIG["tmpb"]))
            ps_e = ctx.enter_context(tc.tile_pool(name="ps_e", bufs=CONFIG["pe"], space="PSUM"))
            ps_d = ctx.enter_context(tc.tile_pool(name="ps_d", bufs=CONFIG["pd"], space="PSUM"))
            ps_c = ctx.enter_context(tc.tile_pool(name="ps_c", bufs=1, space="PSUM"))
            ps_p = ctx.enter_context(tc.tile_pool(name="ps_p", bufs=CONFIG["pp"], space="PSUM"))

            # ---- constants (wexp/t1 now; the rest deferred past x(0) DMA) ----
            wexp_s = const.tile([128, CEXP], BF16, tag="wexp")
            dwdiag_s = const.tile([128, NB * 9 * 128], BF16, tag="dwdiag")
            wctx_s = const.tile([128, NB * CEXP], BF16, tag="wctx")
            wproj_s = const.tile([128, NB * CIN], BF16, tag="wproj")
            t1_s = const.tile([128, NB], F32, tag="t1")
            t2_s = const.tile([128, NB], F32, tag="t2")
            t3_s = const.tile([CIN, 1], F32, tag="t3")
            nc.sync.dma_start(wexp_s[0:CIN, :], wexp_d[:])
            nc.sync.dma_start(wexp_s[CIN : 2 * CIN, :], wexp_d[:])
            nc.sync.dma_start(t1_s[:], t1_d[:])

            # ---- persistent padded h tiles (borders stay zero forever) ----
            NH = 6
            h_tiles = [
                hp.tile([128, PADN], BF16, tag=f"h{i}", name=f"h{i}")
                for i in range(NH)
            ]
            for t in h_tiles:
                nc.gpsimd.memset(t[:].bitcast(mybir.dt.uint16), 0)

            relu = mybir.ActivationFunctionType.Relu
            ident = mybir.ActivationFunctionType.Identity
            addop = mybir.AluOpType.add
            minop = mybir.AluOpType.min

            state = {}

            def front_pre(s):
                x_t = xp.tile([128, P], BF16, tag="x")
                nc.sync.dma_start(x_t[0:CIN, :], x_d[s])
                nc.sync.dma_start(x_t[CIN : 2 * CIN, :], x_d[s])
                if s == 0:
                    # big weights only needed by back(0), one iteration later
                    nc.sync.dma_start(dwdiag_s[:], dwdiag_d[:])
                    nc.sync.dma_start(wctx_s[:], wctx_d[:])
                    nc.sync.dma_start(wproj_s[:], wproj_d[:])
                    nc.sync.dma_start(t2_s[:], t2_d[:])
                    nc.sync.dma_start(t3_s[:], t3_d[:])
                hts = [h_tiles[(s * NB + b) % NH] for b in range(NB)]
                ctxps = [ctxpp.tile([128, NG], F32, tag="ctxp", name=f"ctxp{b}") for b in range(NB)]
                state[s] = dict(x=x_t, hts=hts, ctxps=ctxps)

            def emit_expand_pair(s, b, g):
                """two K=64 expand matmuls run concurrently on PE row-halves
                (tile_position row tiling): chunk g on rows 0-63, chunk g+1
                on rows 64-127."""
                st = state[s]
                gs = [g] if g + 1 >= NG else [g, g + 1]
                pes = []
                for j, gg in enumerate(gs):
                    pe = ps_e.tile([128, CH], F32, tag="pe", name=f"pe{j}")
                    nc.tensor.matmul(
                        pe[:],
                        wexp_s[j * CIN : (j + 1) * CIN, b * 128 : (b + 1) * 128],
                        st["x"][j * CIN : (j + 1) * CIN, gg * CH : (gg + 1) * CH],
                        start=True,
                        stop=True,
                        tile_position=(j * CIN, 0),
                    )
                    pes.append(pe)
                for gg, pe in zip(gs, pes):
                    dst = (
                        st["hts"][b][:, 60 + GP * gg : 60 + GP * gg + GP]
                        .rearrange("p (r w) -> p r w", w=PW)[:, :, 0:W]
                    )
                    src = pe[:].rearrange("p (r w) -> p r w", w=W)
                    nc.scalar.activation(
                        dst, src, relu, bias=t1_s[:, b : b + 1], scale=1.0
                    )
                    # min(.,6) + context partial sum in one DVE pass
                    nc.vector.tensor_scalar(
                        dst, dst, 6.0, None, minop, addop,
                        accum_out=st["ctxps"][b][:, gg : gg + 1],
                    )

            def front_post(s):
                st = state[s]
                ctxs = []
                with nc.allow_low_precision(reason="bf16 ctx vector is plenty"):
                    for b in range(NB):
                        cs = ctxsp.tile([128, 1], BF16, tag="ctxs")
                        nc.vector.tensor_reduce(
                            cs[:], st["ctxps"][b][:], axis=mybir.AxisListType.X,
                            op=addop,
                        )
                        ctxs.append(cs)
                st["ctxs"] = ctxs

            def back_pre(s):
                st = state[s]
                st["b2"] = b2p.tile([128, NB], F32, tag="b2", name="b2")
                st["hb2s"] = [hb2p.tile([128, P], BF16, tag="hb2", name=f"hb2_{b}") for b in range(NB)]
                out_t = outp.tile([CIN, P], BF16, tag="out")
                st["out"] = out_t

            def emit_ctx(s):
                """context-bias matmuls + BN2 bias vector (bias add on DVE so
                the scalar FIFO never waits on instructions behind it)."""
                st = state[s]
                pc = ps_c.tile([128, NB], F32, tag="pc")
                for chn in range(NB):
                    for kb in range(NB):
                        nc.tensor.matmul(
                            pc[:, chn : chn + 1],
                            wctx_s[:, kb * CEXP + chn * 128 : kb * CEXP + (chn + 1) * 128],
                            st["ctxs"][kb][:],
                            start=(kb == 0),
                            stop=(kb == NB - 1),
                        )
                for chn in range(NB):
                    nc.vector.tensor_scalar_add(
                        st["b2"][:, chn : chn + 1],
                        pc[:, chn : chn + 1],
                        t2_s[:, chn : chn + 1],
                    )

            def emit_dw(s, b, g, defer_act=False):
                st = state[s]
                pd = ps_d.tile([128, GP], F32, tag="pd")
                base = 59 + GP * g
                for t, (dy, dx) in enumerate(taps):
                    off = dy * PW + dx
                    nc.tensor.matmul(
                        pd[:],
                        dwdiag_s[:, (b * 9 + t) * 128 : (b * 9 + t + 1) * 128],
                        st["hts"][b][:, base + off : base + off + GP],
                        start=(t == 0),
                        stop=(t == 8),
                    )

                def bn2():
                    pd_v = pd[:].rearrange("p (r w) -> p r w", w=PW)[:, :, 1 : W + 1]
                    dst = st["hb2s"][b][:, g * CH : (g + 1) * CH].rearrange(
                        "p (r w) -> p r w", w=W
                    )
                    nc.scalar.activation(
                        dst, pd_v, relu, bias=st["b2"][:, b : b + 1], scale=1.0
                    )
                    nc.vector.tensor_scalar_min(dst, dst, 6.0)

                if defer_act:
                    return bn2
                bn2()

            def emit_proj_pair(s, g):
                """project groups g (PE col-half A -> PSUM rows 0-63) and g+1
                (col-half B -> rows 64-127) concurrently via col tiling."""
                st = state[s]
                gs = [g] if g + 1 >= NG else [g, g + 1]
                pp = ps_p.tile([128, CH], F32, tag="pp")
                for kb in range(NB):
                    for j, gg in enumerate(gs):
                        # start=True clears has_written for the WHOLE bank, so
                        # only the first matmul into the bank may carry it
                        nc.tensor.matmul(
                            pp[j * CIN : (j + 1) * CIN, :],
                            wproj_s[:, kb * CIN : (kb + 1) * CIN],
                            st["hb2s"][kb][:, gg * CH : (gg + 1) * CH],
                            start=(kb == 0),
                            stop=(kb == NB - 1),
                            tile_position=(0, j * CIN),
                            skip_group_check=True,
                        )
                for j, gg in enumerate(gs):
                    tmp = tmpp.tile([CIN, CH], BF16, tag="tmp", name=f"tmp{j}")
                    nc.scalar.activation(
                        tmp[:], pp[j * CIN : (j + 1) * CIN, :], ident,
                        bias=t3_s[:], scale=1.0,
                    )
                    nc.vector.tensor_add(
                        st["out"][:, gg * CH : (gg + 1) * CH],
                        tmp[:],
                        st["x"][0:CIN, gg * CH : (gg + 1) * CH],
                    )
                    nc.sync.dma_start(
                        out_d[s][:, gg * CH : (gg + 1) * CH],
                        st["out"][:, gg * CH : (gg + 1) * CH],
                    )

            def back_post(s):
                state.pop(s)

            # fused, skewed pipeline: iteration s interleaves expand(s) with
            # depthwise+project(s-1) at (g, b) granularity
            for s in range(NS + 1):
                if s < NS:
                    front_pre(s)
                if s >= 1:
                    back_pre(s - 1)
                if s >= 1:
                    emit_ctx(s - 1)
                for gp in range(0, NG + 1, 2):
                    for b in range(NB):
                        if s < NS and gp < NG:
                            emit_expand_pair(s, b, gp)
                        for gg in (gp, gp + 1):
                            if s >= 1 and gg < NG:
                                emit_dw(s - 1, b, gg)
                        if s >= 1 and b == 1 and gp >= 2:
                            emit_proj_pair(s - 1, gp - 2)
                if s >= 1:
                    emit_proj_pair(s - 1, NG - 1)
                if s < NS:
                    front_post(s)
                if s >= 1:
                    back_post(s - 1)

    nc.compile()
    _CACHE[key] = nc
    return nc


def _prep_weights(w_expand, g1, b1, m1, v1, w_dw, w_ctx, g2, b2, m2, v2,
                  w_proj, g3, b3, m3, v3):
    f = np.float32
    s1 = (g1 / np.sqrt(v1 + EPS)).astype(f)
    t1 = (b1 - m1 * s1).astype(f)
    s2 = (g2 / np.sqrt(v2 + EPS)).astype(f)
    t2 = (b2 - m2 * s2).astype(f)
    s3 = (g3 / np.sqrt(v3 + EPS)).astype(f)
    t3 = (b3 - m3 * s3).astype(f)

    wexp = np.ascontiguousarray((w_expand * s1[:, None]).T.astype(f))  # [64, 384]

    wdw = (w_dw[:, 0] * s2[:, None, None]).reshape(CEXP, 9).astype(f)  # [c, t]
    dwdiag = np.zeros((128, NB * 9, 128), f)
    idx = np.arange(128)
    for b in range(NB):
        for t in range(9):
            dwdiag[idx, b * 9 + t, idx] = wdw[b * 128 : (b + 1) * 128, t]
    dwdiag = np.ascontiguousarray(dwdiag.reshape(128, NB * 9 * 128))

    wctx_f = (w_ctx * s2[:, None] / float(P)).astype(f)  # [o, c]
    wctx = np.ascontiguousarray(
        wctx_f.reshape(CEXP, NB, 128).transpose(2, 1, 0).reshape(128, NB * CEXP)
    )

    wproj_f = (w_proj * s3[:, None]).astype(f)  # [64, 384]
    wproj = np.ascontiguousarray(
        wproj_f.reshape(CIN, NB, 128).transpose(2, 1, 0).reshape(128, NB * CIN)
    )

    t1c = np.ascontiguousarray(t1.reshape(NB, 128).T)
    t2c = np.ascontiguousarray(t2.reshape(NB, 128).T)
    t3c = np.ascontiguousarray(t3.reshape(CIN, 1))
    return dict(
        wexp=wexp.astype(NPBF16), dwdiag=dwdiag.astype(NPBF16),
        wctx=wctx.astype(NPBF16), wproj=wproj.astype(NPBF16),
        t1c=t1c, t2c=t2c, t3c=t3c,
    )


def make_in_maps(inputs):
    x = np.asarray(inputs["x"], dtype=np.float32)
    w = _prep_weights(
        np.asarray(inputs["w_expand"], np.float32),
        np.asarray(inputs["g1"], np.float32), np.asarray(inputs["b1"], np.float32),
        np.asarray(inputs["m1"], np.float32), np.asarray(inputs["v1"], np.float32),
        np.asarray(inputs["w_dw"], np.float32),
        np.asarray(inputs["w_ctx"], np.float32),
        np.asarray(inputs["g2"], np.float32), np.asarray(inputs["b2"], np.float32),
        np.asarray(inputs["m2"], np.float32), np.asarray(inputs["v2"], np.float32),
        np.asarray(inputs["w_proj"], np.float32),
        np.asarray(inputs["g3"], np.float32), np.asarray(inputs["b3"], np.float32),
        np.asarray(inputs["m3"], np.float32), np.asarray(inputs["v3"], np.float32),
    )
    in_maps = []
    for c in range(N_CORES):
        shard = np.ascontiguousarray(
            x[c * NS : (c + 1) * NS].reshape(NS, CIN, P).astype(NPBF16)
        )
        in_maps.append({"x": shard, **w})
    return in_maps


def kernel(**inputs):
    from concourse.bass_utils import run_bass_kernel_spmd

    nc = _build_nc()
    in_maps = make_in_maps(inputs)
    res = run_bass_kernel_spmd(nc, in_maps, list(range(N_CORES))).results
    out = np.concatenate(
        [np.asarray(res[c]["out"]).astype(np.float32) for c in range(N_CORES)],
        axis=0,
    )
    return np.ascontiguousarray(out.reshape(32, CIN, H, W))
